# revision 1
# baseline (speedup 1.0000x reference)
"""Trainium2 Bass kernel for supervised contrastive loss.

reference math (N=16384, d=128, T=0.5):
    sim   = (E @ E.T) / T
    P_i   = sum_{j: lab_j==lab_i, j!=i} exp(sim_ij)
    A_i   = sum_{j != i} exp(sim_ij)
    loss  = mean_{i valid} [log(A_i) - log(P_i)]

Sharding: rows (queries) split across 8 cores; keys (all of E) and
labels replicated.  Per core: bf16 matmul sweep E_q @ E_k.T on the PE,
fused exp+row-sum on the Scalar engine (accum_out), positives by a
fused is_equal*exp masked accumulate on the Vector engine against a
broadcast label row, diagonal term via a Square activation, log-loss
epilogue on-device.  Host sums the 8 partial outputs.
"""

import sys

if "/opt/trn_rl_repo" not in sys.path:
    sys.path.insert(0, "/opt/trn_rl_repo")

import numpy as np
import ml_dtypes

N = 16384
D = 128
NC = 8
RPC = N // NC          # rows per core = 2048
QB = RPC // 128        # query blocks per core = 16
KCH = 2048             # key columns per exp chunk
NKCH = N // KCH        # chunks per row sweep = 8
TEMP = 0.5
BF16 = ml_dtypes.bfloat16

_prog_cache = {}


def _build_program():
    import concourse.bacc as bacc
    import concourse.tile as tile
    import concourse.mybir as mybir

    dt = mybir.dt
    AF = mybir.ActivationFunctionType
    ALU = mybir.AluOpType
    AX = mybir.AxisListType

    nc = bacc.Bacc(
        "TRN2",
        target_bir_lowering=False,
        debug=False,
        enable_asserts=False,
        num_devices=NC,
    )

    eT = nc.dram_tensor("et", [D, N], dt.bfloat16, kind="ExternalInput").ap()
    lab = nc.dram_tensor("lab", [D, N], dt.bfloat16, kind="ExternalInput").ap()
    eqT = nc.dram_tensor("eqt", [D, RPC], dt.bfloat16, kind="ExternalInput").ap()
    erows = nc.dram_tensor("erows", [128, RPC], dt.bfloat16, kind="ExternalInput").ap()
    qlab = nc.dram_tensor("qlab", [128, QB], dt.float32, kind="ExternalInput").ap()
    valid = nc.dram_tensor("valid", [128, QB], dt.float32, kind="ExternalInput").ap()
    out = nc.dram_tensor("loss_rows", [128, QB], dt.float32, kind="ExternalOutput").ap()

    with tile.TileContext(nc) as tc:
        with (
            tc.tile_pool(name="keys", bufs=1) as keys_pool,
            tc.tile_pool(name="qp", bufs=1) as qpool,
            tc.tile_pool(name="ps", bufs=2, space="PSUM") as psum_pool,
            tc.tile_pool(name="ex", bufs=3) as exp_pool,
            tc.tile_pool(name="mk", bufs=2) as msk_pool,
            tc.tile_pool(name="ac", bufs=2) as acc_pool,
            tc.tile_pool(name="fin", bufs=1) as fin_pool,
        ):
            # replicated keys + broadcast labels, chunked 1 MB loads
            kts, lts = [], []
            for j in range(NKCH):
                kt = keys_pool.tile([D, KCH], dt.bfloat16, tag=f"kt{j}")
                nc.sync.dma_start(kt[:], eT[:, j * KCH:(j + 1) * KCH])
                kts.append(kt)
                lt = keys_pool.tile([D, KCH], dt.bfloat16, tag=f"lt{j}")
                nc.sync.dma_start(lt[:], lab[:, j * KCH:(j + 1) * KCH])
                lts.append(lt)

            eq = qpool.tile([D, RPC], dt.bfloat16, tag="eq")
            nc.sync.dma_start(eq[:], eqT[:])
            er = qpool.tile([128, RPC], dt.bfloat16, tag="er")
            nc.sync.dma_start(er[:], erows[:])
            ql = qpool.tile([128, QB], dt.float32, tag="ql")
            nc.sync.dma_start(ql[:], qlab[:])
            vl = qpool.tile([128, QB], dt.float32, tag="vl")
            nc.sync.dma_start(vl[:], valid[:])

            tot = fin_pool.tile([128, QB], dt.float32, tag="tot")
            pos = fin_pool.tile([128, QB], dt.float32, tag="pos")
            ssq = fin_pool.tile([128, QB], dt.float32, tag="ssq")

            for qb in range(QB):
                racc = acc_pool.tile([128, NKCH], dt.float32, tag="racc")
                pacc = acc_pool.tile([128, NKCH], dt.float32, tag="pacc")
                for j in range(NKCH):
                    ps = psum_pool.tile([128, KCH], dt.float32, tag="ps")
                    for t in range(KCH // 512):
                        nc.tensor.matmul(
                            ps[:, t * 512:(t + 1) * 512],
                            eq[:, qb * 128:(qb + 1) * 128],
                            kts[j][:, t * 512:(t + 1) * 512],
                        )
                    ex = exp_pool.tile([128, KCH], dt.bfloat16, tag="ex")
                    nc.scalar.activation(
                        ex[:], ps[:], AF.Exp,
                        scale=1.0 / TEMP,
                        accum_out=racc[:, j:j + 1],
                    )
                    mk = msk_pool.tile([128, KCH], dt.bfloat16, tag="mk")
                    nc.vector.scalar_tensor_tensor(
                        mk[:], lts[j][:], ql[:, qb:qb + 1], ex[:],
                        ALU.is_equal, ALU.mult,
                        accum_out=pacc[:, j:j + 1],
                    )
                nc.vector.tensor_reduce(tot[:, qb:qb + 1], racc[:], AX.X, ALU.add)
                nc.vector.tensor_reduce(pos[:, qb:qb + 1], pacc[:], AX.X, ALU.add)
                sqs = msk_pool.tile([128, 128], dt.bfloat16, tag="sqs")
                nc.scalar.activation(
                    sqs[:], er[:, qb * 128:(qb + 1) * 128], AF.Square,
                    accum_out=ssq[:, qb:qb + 1],
                )

            # epilogue: loss_i = valid_i * (log(A_i) - log(P_i))
            dg = fin_pool.tile([128, QB], dt.float32, tag="dg")
            nc.scalar.activation(dg[:], ssq[:], AF.Exp, scale=1.0 / TEMP)
            af_ = fin_pool.tile([128, QB], dt.float32, tag="af")
            nc.vector.tensor_sub(af_[:], tot[:], dg[:])
            pf = fin_pool.tile([128, QB], dt.float32, tag="pf")
            nc.vector.tensor_sub(pf[:], pos[:], dg[:])
            pfc = fin_pool.tile([128, QB], dt.float32, tag="pfc")
            nc.vector.tensor_scalar_max(pfc[:], pf[:], 1e-30)
            la = fin_pool.tile([128, QB], dt.float32, tag="la")
            nc.scalar.activation(la[:], af_[:], AF.Ln)
            lp = fin_pool.tile([128, QB], dt.float32, tag="lp")
            nc.scalar.activation(lp[:], pfc[:], AF.Ln)
            lo = fin_pool.tile([128, QB], dt.float32, tag="lo")
            nc.vector.tensor_sub(lo[:], la[:], lp[:])
            lm = fin_pool.tile([128, QB], dt.float32, tag="lm")
            nc.vector.tensor_mul(lm[:], lo[:], vl[:])
            nc.sync.dma_start(out[:], lm[:])

    nc.compile()
    return nc


def get_program():
    if "nc" not in _prog_cache:
        _prog_cache["nc"] = _build_program()
    return _prog_cache["nc"]


def make_in_maps(embeddings, partition_labels):
    """Host-side shard/marshal of the full inputs into per-core input maps."""
    emb = np.asarray(embeddings, dtype=np.float32)
    labels = np.asarray(partition_labels)
    lab_f = labels.astype(np.float32)

    eT = np.ascontiguousarray(emb.T).astype(BF16)                      # [128, N]
    lab_row = np.ascontiguousarray(
        np.broadcast_to(lab_f.astype(BF16)[None, :], (D, N))
    )                                                                  # [128, N]
    counts = np.bincount(labels.astype(np.int64), minlength=1)
    valid_all = (counts[labels.astype(np.int64)] >= 2).astype(np.float32)

    in_maps = []
    for c in range(NC):
        rows = emb[c * RPC:(c + 1) * RPC]                              # [RPC, 128]
        eqT_c = np.ascontiguousarray(rows.T).astype(BF16)              # [128, RPC]
        erows_c = np.ascontiguousarray(
            rows.reshape(QB, 128, D).transpose(1, 0, 2).reshape(128, RPC)
        ).astype(BF16)                                                 # [p, qb*128+d]
        qlab_c = np.ascontiguousarray(
            lab_f[c * RPC:(c + 1) * RPC].reshape(QB, 128).T
        ).astype(np.float32)                                           # [128, QB]
        valid_c = np.ascontiguousarray(
            valid_all[c * RPC:(c + 1) * RPC].reshape(QB, 128).T
        ).astype(np.float32)
        in_maps.append({
            "et": eT,
            "lab": lab_row,
            "eqt": eqT_c,
            "erows": erows_c,
            "qlab": qlab_c,
            "valid": valid_c,
        })
    return in_maps, valid_all


def combine(results, valid_all):
    total = np.float64(0.0)
    for r in results:
        total += np.asarray(r["loss_rows"], dtype=np.float64).sum()
    n_valid = int(valid_all.sum())
    if n_valid == 0:
        return np.float32(0.0)
    return np.float32(total / n_valid)


def kernel(embeddings, partition_labels):
    from concourse.bass_utils import run_bass_kernel_spmd

    nc = get_program()
    in_maps, valid_all = make_in_maps(embeddings, partition_labels)
    res = run_bass_kernel_spmd(nc, in_maps, list(range(NC)))
    return combine(res.results, valid_all)
